# revision 1
# baseline (speedup 1.0000x reference)
"""GCN 2-layer kernel for Trainium2 (8 NeuronCores, Bass/Tile).

Strategy:
  - Nodes sharded across 8 cores (degree-balanced snake), 13312 slots/core.
  - Edges partitioned by destination core. Per core, edges are grouped by
    16K-row source sub-ranges of the (AllGathered) feature table so that
    dma_gather's int16 indices stay in range, and packed into 1024-token
    batches (SWDGE descriptor-ring cap) that are destination-unique, so the
    CCE scatter-add has no same-address races within an instruction.
  - Per layer: h = prev @ W (PE), g = dis * h -> fp32 table, AllGather,
    then per batch: dma_gather(table sub-range) -> dma_scatter_add into one
    of 4 DRAM accumulators (round-robin; chains serialize per accumulator).
    Self-loop contribution is folded in by initializing acc0 with g.
  - Tails: z = relu(dis*(acc0+acc1+acc2+acc3) + b1); layer 2 ends with
    log_softmax. All fp32.
"""
import os
import sys

sys.path.insert(0, "/opt/trn_rl_repo")

import numpy as np

N, FIN, H, C = 100000, 128, 64, 64
E = 1600000
NCORES = 8
S = 13312                 # slots per core (104 tiles x 128 = 13 x 1024)
NT = S // 128             # 104 tiles
NG8 = NT // 8             # 13 groups of 8 tiles
GROWS = NCORES * S        # 106496 global table rows
RANGE_W = 16384
NRANGES = (GROWS + RANGE_W - 1) // RANGE_W   # 7
ACC_ROWS = S + 128        # + scratch rows
BATCH = 1024              # tokens per gather/scatter (SWDGE ring cap)
TARGET = 960              # target real tokens per batch
NACC = 4
NQ = 4


def _plan(x, edge_index):
    """Host-side planning. Returns per-core input arrays + batch schedule."""
    x = np.asarray(x, np.float32)
    ei = np.asarray(edge_index, np.int64)
    src, dst = ei[0], ei[1]

    deg = np.bincount(dst, minlength=N).astype(np.float64) + 1.0  # + self loop

    # degree-balanced snake sharding
    order = np.argsort(-deg, kind="stable")
    posn = np.arange(N)
    blk, rem = posn // NCORES, posn % NCORES
    corepat = np.where(blk % 2 == 0, rem, NCORES - 1 - rem).astype(np.int32)
    core_of = np.empty(N, np.int32)
    core_of[order] = corepat

    lrow_of = np.empty(N, np.int64)
    nodes_by_core = []
    for c in range(NCORES):
        nodes_c = order[core_of[order] == c]
        lrow_of[nodes_c] = np.arange(len(nodes_c))
        nodes_by_core.append(nodes_c)

    grow_of = core_of.astype(np.int64) * S + lrow_of

    src_g = grow_of[src]
    dst_c = core_of[dst]
    dst_l = lrow_of[dst]

    core_edges = []
    for c in range(NCORES):
        m = dst_c == c
        es, ed = src_g[m], dst_l[m]
        rr = es // RANGE_W
        core_edges.append((es, ed, rr))

    counts = np.zeros((NCORES, NRANGES), np.int64)
    maxmult = np.zeros((NCORES, NRANGES), np.int64)
    for c in range(NCORES):
        es, ed, rr = core_edges[c]
        for r in range(NRANGES):
            m = rr == r
            counts[c, r] = np.count_nonzero(m)
            if counts[c, r]:
                maxmult[c, r] = np.bincount(ed[m]).max()

    # batches per range: enough for count AND per-dst multiplicity
    B = [int(max(np.ceil(counts[:, r].max() / TARGET), maxmult[:, r].max(), 1))
         for r in range(NRANGES)]

    # assign edges: edge with occurrence j of dst d -> batch (d + j) % B_r;
    # count overflow spills into least-full compatible batches.
    per_core_batches = []   # [c][r] -> list of (gsrc_local, dst) arrays
    nspill = np.zeros((NCORES, NRANGES), np.int64)
    for c in range(NCORES):
        es, ed, rr = core_edges[c]
        byrange = []
        for r in range(NRANGES):
            m = rr == r
            es_r = es[m] - r * RANGE_W
            ed_r = ed[m]
            n = len(ed_r)
            Br = B[r]
            if n == 0:
                byrange.append([(np.zeros(0, np.int64), np.zeros(0, np.int64))
                                for _ in range(Br)])
                continue
            six = np.argsort(ed_r, kind="stable")
            ed_s, es_s = ed_r[six], es_r[six]
            newgrp = np.r_[True, ed_s[1:] != ed_s[:-1]]
            starts = np.flatnonzero(newgrp)
            gix = np.cumsum(newgrp) - 1
            occ = np.arange(n) - starts[gix]
            b_of = (ed_s + occ) % Br
            blists = []
            spill_src = []
            spill_dst = []
            for b in range(Br):
                m2 = np.flatnonzero(b_of == b)
                if len(m2) > BATCH:
                    keep, extra = m2[:BATCH], m2[BATCH:]
                    spill_src.append(es_s[extra])
                    spill_dst.append(ed_s[extra])
                    m2 = keep
                blists.append([es_s[m2], ed_s[m2]])
            if spill_src:
                sp_s = np.concatenate(spill_src)
                sp_d = np.concatenate(spill_dst)
                sets = [set(bl[1].tolist()) for bl in blists]
                for i in range(len(sp_s)):
                    placed = False
                    for b in np.argsort([len(bl[0]) for bl in blists]):
                        if len(blists[b][0]) < BATCH and sp_d[i] not in sets[b]:
                            blists[b][0] = np.append(blists[b][0], sp_s[i])
                            blists[b][1] = np.append(blists[b][1], sp_d[i])
                            sets[b].add(sp_d[i])
                            placed = True
                            break
                    if not placed:
                        blists.append([np.array([sp_s[i]]),
                                       np.array([sp_d[i]])])
                        sets.append({sp_d[i]})
                nspill[c, r] = len(blists) - Br
            byrange.append([(np.asarray(bl[0], np.int64),
                             np.asarray(bl[1], np.int64)) for bl in blists])
        per_core_batches.append(byrange)

    # global batch count per range
    BG = [int(B[r] + nspill[:, r].max()) for r in range(NRANGES)]
    nb_total = sum(BG)
    batch_range = []
    for r in range(NRANGES):
        batch_range += [r] * BG[r]

    def wrap16(v):
        n = len(v)
        a = np.asarray(v, np.int16).reshape(n // 16, 16).T.copy()
        return np.tile(a, (8, 1))

    gidx_all = np.zeros((NCORES, 128, nb_total * (BATCH // 16)), np.int16)
    sidx_all = np.zeros((NCORES, 128, nb_total * (BATCH // 16)), np.int16)
    scratch = (S + (np.arange(BATCH) % 128)).astype(np.int64)
    for c in range(NCORES):
        o = 0
        for r in range(NRANGES):
            bl = per_core_batches[c][r]
            for b in range(BG[r]):
                gt = np.zeros(BATCH, np.int64)
                st = scratch.copy()
                if b < len(bl):
                    gs, gd = bl[b]
                    gt[:len(gs)] = gs
                    st[:len(gd)] = gd
                gidx_all[c][:, o:o + BATCH // 16] = wrap16(gt)
                sidx_all[c][:, o:o + BATCH // 16] = wrap16(st)
                o += BATCH // 16

    # per-core xT, deg arranged by (tile, partition):
    # linear row i  <->  tile 8*(i//1024)+(i%8), partition (i%1024)//8
    lin = np.arange(S)
    tile_id = 8 * (lin // 1024) + lin % 8
    part_id = (lin % 1024) // 8
    col = tile_id * 128 + part_id

    xT_all = np.zeros((NCORES, 128, S), np.float32)
    deg_all = np.ones((NCORES, 128, NT), np.float32)
    for c in range(NCORES):
        nodes_c = nodes_by_core[c]
        n_real = len(nodes_c)
        xT_all[c][:, col[:n_real]] = x[nodes_c].T
        deg_all[c][part_id[:n_real], tile_id[:n_real]] = deg[nodes_c]

    return {
        "nb_total": nb_total,
        "batch_range": batch_range,
        "gidx": gidx_all,
        "sidx": sidx_all,
        "xT": xT_all,
        "deg": deg_all,
        "nodes_by_core": nodes_by_core,
    }


def _build(plan, stage=99):
    import concourse.bacc as bacc
    import concourse.bass as bass
    import concourse.tile as tile
    import concourse.mybir as mybir
    from concourse.masks import make_identity

    f32 = mybir.dt.float32
    i16 = mybir.dt.int16
    AF = mybir.ActivationFunctionType
    ALU = mybir.AluOpType

    nb_total = plan["nb_total"]
    batch_range = plan["batch_range"]
    gidx_cols = plan["gidx"].shape[2]

    nc = bacc.Bacc("TRN2", target_bir_lowering=False, debug=False,
                   num_devices=NCORES, num_swdge_queues=NQ)

    t_xT = nc.dram_tensor("xT", [128, S], f32, kind="ExternalInput")
    t_deg = nc.dram_tensor("deg", [128, NT], f32, kind="ExternalInput")
    t_W1 = nc.dram_tensor("W1", [FIN, H], f32, kind="ExternalInput")
    t_W2 = nc.dram_tensor("W2", [H, C], f32, kind="ExternalInput")
    t_b1 = nc.dram_tensor("b1b", [128, H], f32, kind="ExternalInput")
    t_b2 = nc.dram_tensor("b2b", [128, C], f32, kind="ExternalInput")
    t_gi = nc.dram_tensor("gidx", [128, gidx_cols], i16, kind="ExternalInput")
    t_si = nc.dram_tensor("sidx", [128, gidx_cols], i16, kind="ExternalInput")
    t_y = nc.dram_tensor("y", [S, C], f32, kind="ExternalOutput")

    g1_bounce = nc.dram_tensor("g1_bounce", [S, H], f32, kind="Internal")
    g2_bounce = nc.dram_tensor("g2_bounce", [S, C], f32, kind="Internal")
    g1_table = nc.dram_tensor("g1_table", [GROWS, H], f32, kind="Internal")
    g2_table = nc.dram_tensor("g2_table", [GROWS, C], f32, kind="Internal")
    accs = {}
    for ln in (1, 2):
        for an in range(NACC):
            accs[(ln, an)] = nc.dram_tensor(
                f"acc{ln}{an}", [ACC_ROWS, H], f32, kind="Internal")

    with tile.TileContext(nc) as tc:
        with tc.tile_pool(name="sb", bufs=1) as sbc, \
             tc.tile_pool(name="sbw", bufs=2) as sb, \
             tc.tile_pool(name="sbg", bufs=4) as sbg, \
             tc.tile_pool(name="ps", bufs=3, space="PSUM") as ps:

            W1t = sbc.tile([FIN, H], f32)
            nc.sync.dma_start(out=W1t[:], in_=t_W1[:])
            W2t = sbc.tile([H, C], f32)
            nc.sync.dma_start(out=W2t[:], in_=t_W2[:])
            b1t = sbc.tile([128, H], f32)
            nc.sync.dma_start(out=b1t[:], in_=t_b1[:])
            b2t = sbc.tile([128, C], f32)
            nc.sync.dma_start(out=b2t[:], in_=t_b2[:])
            ident = sbc.tile([128, 128], f32)
            make_identity(nc, ident[:])

            degt = sbc.tile([128, NT], f32)
            nc.sync.dma_start(out=degt[:], in_=t_deg[:])
            sq = sbc.tile([128, NT], f32)
            nc.scalar.sqrt(sq[:], degt[:])
            dis = sbc.tile([128, NT], f32)
            nc.vector.reciprocal(dis[:], sq[:])

            zt = sbc.tile([128, 840], f32)
            nc.vector.memset(zt[:], 0.0)
            KREP = int(os.environ.get("KREP", "1"))
            for _rep in range(KREP):
              # zero accs 1..3 (acc0 gets the g init = self-loop term)
              for ln in (1, 2):
                for an in range(1, NACC):
                    a = accs[(ln, an)]
                    for k in range(8):
                        nc.sync.dma_start(
                            out=a[k * 1680:(k + 1) * 1680, :], in_=zt[:])

              # ---------------- prep: g1 = dis * (x @ W1) ----------------
              for Gi in range(NG8):
                  xc = sbg.tile([128, 1024], f32, tag="xc")
                  nc.sync.dma_start(out=xc[:],
                                    in_=t_xT[:, Gi * 1024:(Gi + 1) * 1024])
                  g8 = sbg.tile([128, 8, H], f32, tag="g8")
                  for j in range(8):
                      t = 8 * Gi + j
                      h1T = ps.tile([H, 128], f32, tag="pT")
                      nc.tensor.matmul(out=h1T[:], lhsT=W1t[:],
                                       rhs=xc[:, j * 128:(j + 1) * 128],
                                       start=True, stop=True)
                      h1Ts = sb.tile([H, 128], f32, tag="h1Ts")
                      nc.scalar.copy(h1Ts[:], h1T[:])
                      h1 = ps.tile([128, H], f32, tag="pN")
                      nc.tensor.transpose(out=h1[:], in_=h1Ts[:],
                                          identity=ident[0:H, 0:H])
                      nc.vector.tensor_scalar(
                          out=g8[:, j, :], in0=h1[:],
                          scalar1=dis[:, t:t + 1], scalar2=None,
                          op0=ALU.mult)
                  nc.sync.dma_start(
                      out=g1_bounce[Gi * 1024:(Gi + 1) * 1024, :], in_=g8[:])
                  nc.scalar.dma_start(
                      out=accs[(1, 0)][Gi * 1024:(Gi + 1) * 1024, :], in_=g8[:])

              if stage >= 2:
                  nc.gpsimd.collective_compute(
                      "AllGather", mybir.AluOpType.bypass,
                      replica_groups=[list(range(NCORES))],
                      ins=[g1_bounce[:]], outs=[g1_table[:]])

              # ---------------- edge phase ----------------
              def edge_phase(table, ln):
                  o = 0
                  for bi in range(nb_total):
                      r = batch_range[bi]
                      r1 = min((r + 1) * RANGE_W, GROWS)
                      q = bi % NQ
                      gi = sbg.tile([128, BATCH // 16], i16, tag="gi")
                      nc.sync.dma_start(out=gi[:], in_=t_gi[:, o:o + BATCH // 16])
                      si = sbg.tile([128, BATCH // 16], i16, tag="si")
                      nc.scalar.dma_start(out=si[:],
                                          in_=t_si[:, o:o + BATCH // 16])
                      buf = sbg.tile([128, BATCH // 128, H], f32, tag="buf")
                      nc.gpsimd.dma_gather(
                          out_ap=buf[:],
                          in_ap=table[r * RANGE_W:r1, :],
                          idxs_ap=gi[:],
                          num_idxs=BATCH,
                          num_idxs_reg=BATCH,
                          elem_size=H,
                          queue_num=q,
                      )
                      nc.gpsimd.dma_scatter_add(
                          out_ap=accs[(ln, bi % NACC)][:],
                          in_ap=buf[:],
                          idxs_ap=si[:],
                          num_idxs=BATCH,
                          num_idxs_reg=BATCH,
                          elem_size=H,
                          queue_num=q,
                      )
                      o += BATCH // 16

              if stage >= 3:
                  edge_phase(g1_table, 1)

              # ------------- layer-1 tails: z=relu(dis*s+b1); g2=dis*(z@W2)
              if stage >= 4:
                  for Gi in range(NG8):
                      a0 = sbg.tile([128, 8, H], f32, tag="a0")
                      nc.sync.dma_start(
                          out=a0[:],
                          in_=accs[(1, 0)][Gi * 1024:(Gi + 1) * 1024, :])
                      a1 = sbg.tile([128, 8, H], f32, tag="a1")
                      nc.sync.dma_start(
                          out=a1[:],
                          in_=accs[(1, 1)][Gi * 1024:(Gi + 1) * 1024, :])
                      a2 = sbg.tile([128, 8, H], f32, tag="a2")
                      nc.scalar.dma_start(
                          out=a2[:],
                          in_=accs[(1, 2)][Gi * 1024:(Gi + 1) * 1024, :])
                      a3 = sbg.tile([128, 8, H], f32, tag="a3")
                      nc.scalar.dma_start(
                          out=a3[:],
                          in_=accs[(1, 3)][Gi * 1024:(Gi + 1) * 1024, :])
                      t01 = sbg.tile([128, 8 * H], f32, tag="t01")
                      nc.gpsimd.tensor_tensor(
                          out=t01[:], in0=a0[:], in1=a1[:], op=ALU.add)
                      t23 = sbg.tile([128, 8 * H], f32, tag="t23")
                      nc.gpsimd.tensor_tensor(
                          out=t23[:], in0=a2[:], in1=a3[:], op=ALU.add)
                      s8 = sbg.tile([128, 8 * H], f32, tag="s8")
                      nc.vector.tensor_tensor(out=s8[:], in0=t01[:], in1=t23[:],
                                              op=ALU.add)
                      g8 = sbg.tile([128, 8, H], f32, tag="g28")
                      for j in range(8):
                          t = 8 * Gi + j
                          zp = sb.tile([128, H], f32, tag="zp")
                          nc.vector.tensor_scalar(
                              out=zp[:], in0=s8[:, j * H:(j + 1) * H],
                              scalar1=dis[:, t:t + 1],
                              scalar2=None, op0=ALU.mult)
                          nc.vector.tensor_tensor(out=zp[:], in0=zp[:],
                                                  in1=b1t[:], op=ALU.add)
                          z = sb.tile([128, H], f32, tag="z")
                          nc.scalar.activation(z[:], zp[:], AF.Relu)
                          zT = ps.tile([H, 128], f32, tag="pT")
                          nc.tensor.transpose(out=zT[:], in_=z[:],
                                              identity=ident[:])
                          zTs = sb.tile([H, 128], f32, tag="zTs")
                          nc.scalar.copy(zTs[:], zT[:])
                          h2T = ps.tile([C, 128], f32, tag="pT")
                          nc.tensor.matmul(out=h2T[:], lhsT=W2t[:], rhs=zTs[:],
                                           start=True, stop=True)
                          h2Ts = sb.tile([C, 128], f32, tag="h2Ts")
                          nc.scalar.copy(h2Ts[:], h2T[:])
                          h2 = ps.tile([128, C], f32, tag="pN")
                          nc.tensor.transpose(out=h2[:], in_=h2Ts[:],
                                              identity=ident[0:C, 0:C])
                          nc.vector.tensor_scalar(
                              out=g8[:, j, :], in0=h2[:],
                              scalar1=dis[:, t:t + 1], scalar2=None,
                              op0=ALU.mult)
                      nc.sync.dma_start(
                          out=g2_bounce[Gi * 1024:(Gi + 1) * 1024, :], in_=g8[:])
                      nc.scalar.dma_start(
                          out=accs[(2, 0)][Gi * 1024:(Gi + 1) * 1024, :],
                          in_=g8[:])

              if stage >= 5:
                  nc.gpsimd.collective_compute(
                      "AllGather", mybir.AluOpType.bypass,
                      replica_groups=[list(range(NCORES))],
                      ins=[g2_bounce[:]], outs=[g2_table[:]])

              if stage >= 6:
                  edge_phase(g2_table, 2)

              # ---------------- layer-2 tails: log_softmax ----------------
              for Gi in range(NG8):
                  a0 = sbg.tile([128, 8, C], f32, tag="a0")
                  nc.sync.dma_start(
                      out=a0[:], in_=accs[(2, 0)][Gi * 1024:(Gi + 1) * 1024, :])
                  a1 = sbg.tile([128, 8, C], f32, tag="a1")
                  nc.sync.dma_start(
                      out=a1[:], in_=accs[(2, 1)][Gi * 1024:(Gi + 1) * 1024, :])
                  a2 = sbg.tile([128, 8, C], f32, tag="a2")
                  nc.scalar.dma_start(
                      out=a2[:], in_=accs[(2, 2)][Gi * 1024:(Gi + 1) * 1024, :])
                  a3 = sbg.tile([128, 8, C], f32, tag="a3")
                  nc.scalar.dma_start(
                      out=a3[:], in_=accs[(2, 3)][Gi * 1024:(Gi + 1) * 1024, :])
                  t01 = sbg.tile([128, 8 * C], f32, tag="t01")
                  nc.gpsimd.tensor_tensor(
                      out=t01[:], in0=a0[:], in1=a1[:], op=ALU.add)
                  t23 = sbg.tile([128, 8 * C], f32, tag="t23")
                  nc.gpsimd.tensor_tensor(
                      out=t23[:], in0=a2[:], in1=a3[:], op=ALU.add)
                  s8 = sbg.tile([128, 8 * C], f32, tag="s8")
                  nc.vector.tensor_tensor(out=s8[:], in0=t01[:], in1=t23[:],
                                          op=ALU.add)
                  y8 = sbg.tile([128, 8, C], f32, tag="y8")
                  for j in range(8):
                      t = 8 * Gi + j
                      lg = sb.tile([128, C], f32, tag="lg")
                      nc.vector.tensor_scalar(
                          out=lg[:], in0=s8[:, j * C:(j + 1) * C],
                          scalar1=dis[:, t:t + 1], scalar2=None, op0=ALU.mult)
                      nc.vector.tensor_tensor(out=lg[:], in0=lg[:], in1=b2t[:],
                                              op=ALU.add)
                      nmax = sb.tile([128, 1], f32, tag="nmax")
                      nc.vector.tensor_reduce(
                          out=nmax[:], in_=lg[:], axis=mybir.AxisListType.X,
                          op=ALU.max, negate=True)
                      ex = sb.tile([128, C], f32, tag="ex")
                      sume = sb.tile([128, 1], f32, tag="sume")
                      nc.scalar.activation(ex[:], lg[:], AF.Exp,
                                           bias=nmax[:], scale=1.0,
                                           accum_out=sume[:])
                      lse = sb.tile([128, 1], f32, tag="lse")
                      nc.scalar.activation(lse[:], sume[:], AF.Ln)
                      cc = sb.tile([128, 1], f32, tag="cc")
                      nc.vector.tensor_tensor(out=cc[:], in0=nmax[:],
                                              in1=lse[:], op=ALU.subtract)
                      nc.vector.tensor_scalar(
                          out=y8[:, j, :], in0=lg[:], scalar1=cc[:],
                          scalar2=None, op0=ALU.add)
                  nc.sync.dma_start(
                      out=t_y[Gi * 1024:(Gi + 1) * 1024, :], in_=y8[:])

    nc.compile()
    return nc


def _run(inputs, trace=False):
    import concourse.bass_utils as bass_utils

    x = np.asarray(inputs["x"], np.float32)
    W1 = np.asarray(inputs["W1"], np.float32)
    b1 = np.asarray(inputs["b1"], np.float32)
    W2 = np.asarray(inputs["W2"], np.float32)
    b2 = np.asarray(inputs["b2"], np.float32)

    plan = _plan(x, inputs["edge_index"])
    nc = _build(plan, stage=int(os.environ.get("KSTAGE", "99")))

    b1b = np.tile(b1[None, :], (128, 1)).astype(np.float32)
    b2b = np.tile(b2[None, :], (128, 1)).astype(np.float32)

    in_maps = []
    for c in range(NCORES):
        in_maps.append({
            "xT": plan["xT"][c],
            "deg": plan["deg"][c],
            "W1": W1, "W2": W2, "b1b": b1b, "b2b": b2b,
            "gidx": plan["gidx"][c],
            "sidx": plan["sidx"][c],
        })

    res = bass_utils.run_bass_kernel_spmd(
        nc, in_maps, core_ids=list(range(NCORES)), trace=trace)

    out = np.empty((N, C), np.float32)
    for c in range(NCORES):
        yc = np.asarray(res.results[c]["y"], np.float32)
        nodes_c = plan["nodes_by_core"][c]
        out[nodes_c] = yc[:len(nodes_c)]
    return out, res


def kernel(**inputs):
    out, _ = _run(inputs, trace=False)
    return out



# revision 11
# speedup vs baseline: 1.1264x; 1.1264x over previous
"""GCN 2-layer kernel for Trainium2 (8 NeuronCores, Bass/Tile), v2.

Strategy (vs v1 which used dma_scatter_add into DRAM accumulators):
  - Nodes sharded across 8 cores (degree-balanced snake), S=13312 slots/core
    = 104 blocks of 128 nodes.
  - Per layer: a node table g (g1 = dis*(x@W1), g2 = dis*z) is computed
    per-shard, AllGathered to a full [106496, 64] fp32 DRAM table.
    NOTE: the layer-2 W2 matmul is moved AFTER aggregation (linearity),
    so the layer-2 table is just dis*z.
  - Edge phase: edges (plus self-loops) are grouped by destination block
    (128 dsts) and source range (16K rows, int16 gather index limit), with
    token counts padded to a cross-core-uniform static schedule.
    dma_gather pulls source rows (256B each) into SBUF tiles of
    [128 tokens, 64 feats]; the TensorEngine segment-reduces each tile into
    a feature-major PSUM accumulator [64, 1536] (24-block superblock pairs
    on partition halves) via tile-built 0/1 matrices
    M[token, dst_off] = (dstoff[token] == iota), streamed as matmul rhs.
    PSUM banks are cleared by outer-product zero-matmuls (start=True),
    real matmuls accumulate with start=False.
  - Tails per 128-dst block: transpose to node-major via PE, then
    layer 1: z~ = dis*relu(dis*agg + b1) -> layer-2 table;
    layer 2: h2 = (agg @ W2), y = log_softmax(dis*h2 + b2).
"""
import os
import sys

sys.path.insert(0, "/opt/trn_rl_repo")

import numpy as np

N, FIN, HD, C = 100000, 128, 64, 64
E = 1600000
NCORES = 8
S = 13312                  # slots per core
NT = S // 128              # 104 dst blocks
GROWS = NCORES * S         # 106496 global table rows
RANGE_W = int(os.environ.get("KRW", "16384"))  # gather index range (int16)
NR = (GROWS + RANGE_W - 1) // RANGE_W
PAIRW = 24                 # dst blocks per superblock-pair (2 halves x 12)
NSBP = (NT + PAIRW - 1) // PAIRW          # 5 (24,24,24,24,8)
HALF_BLKS = PAIRW // 2     # 12
HALF_COLS = HALF_BLKS * 128  # 1536 psum cols per half (3 banks)
BATCH = 1024               # max tokens per gather
NQ = 4


def _plan(x, edge_index):
    """Host-side planning. Returns per-core arrays + a uniform schedule."""
    x = np.asarray(x, np.float32)
    ei = np.asarray(edge_index, np.int64)
    src, dst = ei[0], ei[1]

    deg = np.bincount(dst, minlength=N).astype(np.float64) + 1.0  # + self loop
    dis_f = (1.0 / np.sqrt(deg)).astype(np.float32)

    # degree-balanced snake sharding
    order = np.argsort(-deg, kind="stable")
    posn = np.arange(N)
    blk, rem = posn // NCORES, posn % NCORES
    corepat = np.where(blk % 2 == 0, rem, NCORES - 1 - rem).astype(np.int32)
    core_of = np.empty(N, np.int32)
    core_of[order] = corepat

    lrow_of = np.empty(N, np.int64)
    nodes_by_core = []
    for c in range(NCORES):
        nodes_c = order[core_of[order] == c]
        lrow_of[nodes_c] = np.arange(len(nodes_c))
        nodes_by_core.append(nodes_c)

    grow_of = core_of.astype(np.int64) * S + lrow_of

    # per-core token lists: real edges only (self loops get dedicated
    # "self" groups gathering from the local bounce tensor)
    tok_src = []   # global row of source
    tok_dst = []   # local row of dest
    dst_core = core_of[dst]
    for c in range(NCORES):
        m = dst_core == c
        tok_src.append(grow_of[src[m]])
        tok_dst.append(lrow_of[dst[m]])

    # counts per (core, block, range)
    cnt = np.zeros((NCORES, NT, NR), np.int64)
    for c in range(NCORES):
        b = tok_dst[c] >> 7
        r = tok_src[c] // RANGE_W
        np.add.at(cnt[c], (b, r), 1)

    # uniform padded tile counts per (block, range)
    ntok = cnt.max(axis=0)                       # [NT, NR]
    ntiles_br = -(-ntok // 128)                  # ceil to 128-token tiles

    # schedule: for each sbp: a self group (r = -1, one tile per block,
    # gathered from the bounce tensor), then per-range edge groups.
    # Tiles in block-major order; gathers chunk them by 8 (last partial).
    sched = []            # per (sbp, r): dict(tiles=[(b)], ngather)
    total_tiles = 0
    for sbp in range(NSBP):
        b0, b1 = sbp * PAIRW, min((sbp + 1) * PAIRW, NT)
        groups = [(-1, list(range(b0, b1)))]
        for r in range(NR):
            tiles = []
            for b in range(b0, b1):
                tiles += [b] * int(ntiles_br[b, r])
            groups.append((r, tiles))
        for r, tiles in groups:
            ng = -(-len(tiles) // 8) if tiles else 0
            sched.append({
                "sbp": sbp, "r": r, "tiles": tiles, "ngather": ng,
            })
            total_tiles += len(tiles)

    ngather_total = sum(g["ngather"] for g in sched)
    nmm_total = total_tiles

    # build per-core gidx + dstoff arrays following the schedule
    def wrap16(v):
        n = len(v)
        a = np.asarray(v, np.int16).reshape(n // 16, 16).T.copy()
        return np.tile(a, (8, 1))

    gidx_all = np.zeros((NCORES, 128, ngather_total * (BATCH // 16)), np.int16)
    doff_all = np.full((NCORES, 128, max(nmm_total, 1)), -1.0, np.float32)

    for c in range(NCORES):
        b_of = tok_dst[c] >> 7
        r_of = tok_src[c] // RANGE_W
        # bucket tokens by (block, range)
        key = b_of * NR + r_of
        osort = np.argsort(key, kind="stable")
        ts_s, td_s = tok_src[c][osort], tok_dst[c][osort]
        key_s = key[osort]
        starts = np.searchsorted(key_s, np.arange(NT * NR))
        ends = np.searchsorted(key_s, np.arange(NT * NR) + 1)

        g_cursor = 0   # in gathers
        m_cursor = 0   # in mms (= tiles)
        for grp in sched:
            r = grp["r"]
            tiles = grp["tiles"]
            if not tiles:
                continue
            # assemble this group's token stream (tile-major, 128 per tile)
            n_tok = len(tiles) * 128
            idx_loc = np.zeros(n_tok, np.int64)   # gather idx within range
            dof = np.full(n_tok, -1.0, np.float32)
            if r < 0:
                # self group: one tile per block, idx = own local row
                for i, b in enumerate(tiles):
                    idx_loc[i * 128:(i + 1) * 128] = b * 128 + np.arange(128)
                    dof[i * 128:(i + 1) * 128] = np.arange(128, dtype=np.float32)
            else:
                # fill per block
                tpos = 0
                tiles_arr = np.asarray(tiles)
                for b in np.unique(tiles_arr):
                    k = b * NR + r
                    s0, s1 = starts[k], ends[k]
                    cnt_b = s1 - s0
                    nt_b = int(np.count_nonzero(tiles_arr == b))
                    # tokens for this block go into its nt_b tiles
                    off = tpos * 128
                    idx_loc[off:off + cnt_b] = ts_s[s0:s1] - r * RANGE_W
                    dof[off:off + cnt_b] = (
                        td_s[s0:s1] - b * 128).astype(np.float32)
                    tpos += nt_b
            # scatter into gidx (per gather chunk) and dstoff (per tile)
            n_tiles = len(tiles)
            for g in range(grp["ngather"]):
                t0, t1 = g * 8, min((g + 1) * 8, n_tiles)
                nidx = (t1 - t0) * 128
                seg = idx_loc[t0 * 128: t0 * 128 + nidx]
                col0 = (g_cursor + g) * (BATCH // 16)
                w = wrap16(np.pad(seg, (0, BATCH - nidx)))
                gidx_all[c][:, col0:col0 + BATCH // 16] = w
            doff_all[c][:, m_cursor:m_cursor + n_tiles] = (
                dof.reshape(n_tiles, 128).T)
            g_cursor += grp["ngather"]
            m_cursor += n_tiles

    # per-core xT, dis with p = row%128, t = row//128
    xT_all = np.zeros((NCORES, 128, S), np.float32)
    dis_all = np.ones((NCORES, 128, NT), np.float32)
    for c in range(NCORES):
        nodes_c = nodes_by_core[c]
        n_real = len(nodes_c)
        lin = np.arange(n_real)
        xT_all[c][:, lin] = x[nodes_c].T
        dis_all[c][lin % 128, lin // 128] = dis_f[nodes_c]

    # consts tile: [128, 640]: cols 0:128 iota, 128:640 zeros
    consts = np.zeros((128, 640), np.float32)
    consts[:, 0:128] = np.arange(128, dtype=np.float32)[None, :]

    return {
        "sched": sched,
        "ngather_total": ngather_total,
        "nmm_total": nmm_total,
        "gidx": gidx_all,
        "doff": doff_all,
        "xT": xT_all,
        "dis": dis_all,
        "consts": consts,
        "nodes_by_core": nodes_by_core,
    }


def _build(plan, stage=99):
    import concourse.bacc as bacc
    import concourse.bass as bass
    import concourse.tile as tile
    import concourse.mybir as mybir

    f32 = mybir.dt.float32
    i16 = mybir.dt.int16
    AF = mybir.ActivationFunctionType
    ALU = mybir.AluOpType

    sched = plan["sched"]
    ngather_total = plan["ngather_total"]
    nmm_total = plan["nmm_total"]
    gcols = ngather_total * (BATCH // 16)
    max_ng = max(g["ngather"] for g in sched)

    nc = bacc.Bacc("TRN2", target_bir_lowering=False, debug=False,
                   num_devices=NCORES, num_swdge_queues=NQ)

    t_xT = nc.dram_tensor("xT", [128, S], f32, kind="ExternalInput")
    t_dis = nc.dram_tensor("dis", [128, NT], f32, kind="ExternalInput")
    t_W1 = nc.dram_tensor("W1", [FIN, HD], f32, kind="ExternalInput")
    t_W2b = nc.dram_tensor("W2b", [128, C], f32, kind="ExternalInput")
    t_b1 = nc.dram_tensor("b1b", [128, HD], f32, kind="ExternalInput")
    t_b2 = nc.dram_tensor("b2b", [128, C], f32, kind="ExternalInput")
    t_id2 = nc.dram_tensor("id2", [128, 64], f32, kind="ExternalInput")
    t_gi = nc.dram_tensor("gidx", [128, gcols], i16, kind="ExternalInput")
    t_do = nc.dram_tensor("doff", [128, nmm_total], f32, kind="ExternalInput")
    t_cn = nc.dram_tensor("consts", [128, 640], f32, kind="ExternalInput")
    t_y = nc.dram_tensor("y", [S, C], f32, kind="ExternalOutput")

    g1_bounce = nc.dram_tensor("g1_bounce", [S, HD], f32, kind="Internal")
    g2_bounce = nc.dram_tensor("g2_bounce", [S, HD], f32, kind="Internal")
    g1_table = nc.dram_tensor("g1_table", [GROWS, HD], f32, kind="Internal",
                              addr_space="Shared")
    g2_table = nc.dram_tensor("g2_table", [GROWS, HD], f32, kind="Internal",
                              addr_space="Shared")

    with tile.TileContext(nc) as tc:
        with tc.tile_pool(name="sbc", bufs=1) as sbc, \
             tc.tile_pool(name="sbx", bufs=2) as sbx, \
             tc.tile_pool(name="sbg", bufs=6) as sbg, \
             tc.tile_pool(name="sbm", bufs=4) as sbm, \
             tc.tile_pool(name="sbi", bufs=2) as sbi, \
             tc.tile_pool(name="sbt", bufs=3) as sbt, \
             tc.tile_pool(name="psa", bufs=2, space="PSUM") as psa, \
             tc.tile_pool(name="pst", bufs=2, space="PSUM") as pst:

            W1t = sbc.tile([FIN, HD], f32)
            nc.sync.dma_start(out=W1t[:], in_=t_W1[:])
            W2bt = sbc.tile([128, C], f32)
            nc.sync.dma_start(out=W2bt[:], in_=t_W2b[:])
            b1t = sbc.tile([128, HD], f32)
            nc.sync.dma_start(out=b1t[:], in_=t_b1[:])
            b2t = sbc.tile([128, C], f32)
            nc.sync.dma_start(out=b2t[:], in_=t_b2[:])
            id2t = sbc.tile([128, 64], f32)
            nc.sync.dma_start(out=id2t[:], in_=t_id2[:])
            cons = sbc.tile([128, 640], f32)
            nc.sync.dma_start(out=cons[:], in_=t_cn[:])
            dist = sbc.tile([128, NT], f32)
            nc.sync.dma_start(out=dist[:], in_=t_dis[:])

            iota = cons[:, 0:128]
            zrow = cons[0:1, 512:640]      # zeros [1, 128]
            zrhs = cons[0:1, 128:640]      # zeros [1, 512]

            # ---------------- layer-1 prep: g1 = dis * (x @ W1) -------------
            NCHUNK = S // 1024             # 13
            for ch in range(NCHUNK):
                xc = sbx.tile([128, 1024], f32, tag="xc")
                nc.sync.dma_start(out=xc[:],
                                  in_=t_xT[:, ch * 1024:(ch + 1) * 1024])
                for j in range(8):
                    t = 8 * ch + j
                    pp = pst.tile([128, 128], f32, tag="pp")
                    nc.tensor.matmul(out=pp[:, 0:HD],
                                     lhsT=xc[:, j * 128:(j + 1) * 128],
                                     rhs=W1t[:], start=True, stop=True)
                    g1 = sbt.tile([128, HD], f32, tag="g1")
                    nc.vector.tensor_scalar(
                        out=g1[:], in0=pp[:, 0:HD],
                        scalar1=dist[:, t:t + 1], scalar2=None, op0=ALU.mult)
                    nc.sync.dma_start(
                        out=g1_bounce[t * 128:(t + 1) * 128, :], in_=g1[:])

            if stage >= 2:
                nc.gpsimd.collective_compute(
                    "AllGather", mybir.AluOpType.bypass,
                    replica_groups=[list(range(NCORES))],
                    ins=[g1_bounce[:]], outs=[g1_table[:]])

            # ---------------- edge phase ----------------
            def edge_phase(table, bounce, layer):
                g_cursor = 0
                m_cursor = 0
                gq = 0
                for sbp in range(NSBP):
                    b0 = sbp * PAIRW
                    acc = psa.tile([128, HALF_COLS], f32, tag="acc")
                    # clear all 3 banks (both halves at once)
                    for seg in range(3):
                        nc.tensor.matmul(
                            out=acc[:, seg * 512:(seg + 1) * 512],
                            lhsT=zrow[:], rhs=zrhs[:],
                            start=True, stop=False, skip_group_check=True)
                    # find last mm per bank to set stop
                    grp_list = [g for g in sched if g["sbp"] == sbp]
                    last_of_seg = {}
                    mm_idx = 0
                    for grp in grp_list:
                        for b in grp["tiles"]:
                            lb = b - b0
                            seg = ((lb % HALF_BLKS) * 128) // 512
                            last_of_seg[seg] = mm_idx
                            mm_idx += 1
                    mm_idx = 0
                    for grp in grp_list:
                        r = grp["r"]
                        tiles = grp["tiles"]
                        if not tiles:
                            continue
                        if r < 0:
                            src_ap = bounce[:]
                        else:
                            r1 = min((r + 1) * RANGE_W, GROWS)
                            src_ap = table[r * RANGE_W:r1, :]
                        n_tiles = len(tiles)
                        ng = grp["ngather"]
                        gi = sbi.tile([128, max_ng * (BATCH // 16)], i16,
                                      tag="gi")
                        col0 = g_cursor * (BATCH // 16)
                        nc.sync.dma_start(
                            out=gi[:, 0:ng * (BATCH // 16)],
                            in_=t_gi[:, col0:col0 + ng * (BATCH // 16)])
                        for g in range(ng):
                            t0 = g * 8
                            t1 = min(t0 + 8, n_tiles)
                            nt8 = t1 - t0
                            nidx = nt8 * 128
                            gcol = g * (BATCH // 16)
                            buf = sbg.tile([128, 8, HD], f32, tag="buf")
                            nc.gpsimd.dma_gather(
                                out_ap=buf[:, 0:nt8, :],
                                in_ap=src_ap,
                                idxs_ap=gi[:, gcol:gcol + nidx // 16],
                                num_idxs=nidx,
                                num_idxs_reg=nidx,
                                elem_size=HD,
                                queue_num=gq % NQ,
                            )
                            gq += 1
                            # batched M build for this chunk's tiles
                            mm0 = m_cursor + t0
                            M = sbm.tile([128, 8, 128], f32, tag="M")
                            do_sl = dof_sb[:, mm0 - m_base:mm0 - m_base + nt8]
                            nc.vector.tensor_tensor(
                                out=M[:, 0:nt8, :],
                                in0=iota.unsqueeze(1).broadcast_to(
                                    [128, nt8, 128]),
                                in1=do_sl.unsqueeze(2).broadcast_to(
                                    [128, nt8, 128]),
                                op=ALU.is_equal)
                            for j in range(nt8):
                                b = tiles[t0 + j]
                                lb = b - b0
                                half = lb // HALF_BLKS
                                col = (lb % HALF_BLKS) * 128
                                nc.tensor.matmul(
                                    out=acc[64 * half:64 * half + 64,
                                            col:col + 128],
                                    lhsT=buf[:, j, :],
                                    rhs=M[:, j, :],
                                    start=False,
                                    stop=(mm_idx == last_of_seg.get(
                                        (col // 512), -2)),
                                    skip_group_check=True,
                                    tile_position=(0, 64 * half))
                                mm_idx += 1
                        g_cursor += grp["ngather"]
                        m_cursor += n_tiles
                    # ---------------- tails for this sbp ----------------
                    nblk = min(PAIRW, NT - b0)
                    for lb in range(nblk):
                        b = b0 + lb
                        half = lb // HALF_BLKS
                        col = (lb % HALF_BLKS) * 128
                        hs, he = 64 * half, 64 * half + 64
                        pt = sbt.tile([128, 128], f32, tag="pt")
                        nc.scalar.copy(pt[hs:he, :], acc[hs:he, col:col + 128])
                        if layer == 1:
                            tr = pst.tile([128, 128], f32, tag="pp")
                            nc.tensor.transpose(out=tr[:, 0:64],
                                                in_=pt[hs:he, :],
                                                identity=id2t[hs:he, :])
                            t1v = sbt.tile([128, HD], f32, tag="t1v")
                            nc.vector.scalar_tensor_tensor(
                                out=t1v[:], in0=tr[:, 0:64],
                                scalar=dist[:, b:b + 1],
                                in1=b1t[:], op0=ALU.mult, op1=ALU.add)
                            zt = sbt.tile([128, HD], f32, tag="zt")
                            nc.scalar.activation(zt[:], t1v[:], AF.Relu)
                            zs = sbt.tile([128, HD], f32, tag="zs")
                            nc.vector.tensor_scalar(
                                out=zs[:], in0=zt[:],
                                scalar1=dist[:, b:b + 1], scalar2=None,
                                op0=ALU.mult)
                            nc.sync.dma_start(
                                out=g2_bounce[b * 128:(b + 1) * 128, :],
                                in_=zs[:])
                        else:
                            h2T = pst.tile([128, 128], f32, tag="pp")
                            nc.tensor.matmul(out=h2T[0:64, :],
                                             lhsT=W2bt[hs:he, :],
                                             rhs=pt[hs:he, :],
                                             start=True, stop=True,
                                             tile_position=(64 * half, 0))
                            h2Ts = sbt.tile([128, 128], f32, tag="h2Ts")
                            nc.scalar.copy(h2Ts[0:64, :], h2T[0:64, :])
                            h2 = pst.tile([128, 128], f32, tag="pp")
                            nc.tensor.transpose(out=h2[:, 0:64],
                                                in_=h2Ts[0:64, :],
                                                identity=id2t[0:64, :])
                            lg = sbt.tile([128, C], f32, tag="lg")
                            nc.vector.scalar_tensor_tensor(
                                out=lg[:], in0=h2[:, 0:64],
                                scalar=dist[:, b:b + 1],
                                in1=b2t[:], op0=ALU.mult, op1=ALU.add)
                            nmax = sbt.tile([128, 1], f32, tag="nmax")
                            nc.vector.tensor_reduce(
                                out=nmax[:], in_=lg[:],
                                axis=mybir.AxisListType.X,
                                op=ALU.max, negate=True)
                            ex = sbt.tile([128, C], f32, tag="ex")
                            sume = sbt.tile([128, 1], f32, tag="sume")
                            nc.scalar.activation(ex[:], lg[:], AF.Exp,
                                                 bias=nmax[:], scale=1.0,
                                                 accum_out=sume[:])
                            lse = sbt.tile([128, 1], f32, tag="lse")
                            nc.scalar.activation(lse[:], sume[:], AF.Ln)
                            cc = sbt.tile([128, 1], f32, tag="cc")
                            nc.vector.tensor_tensor(out=cc[:], in0=nmax[:],
                                                    in1=lse[:],
                                                    op=ALU.subtract)
                            yt = sbt.tile([128, C], f32, tag="yt")
                            nc.vector.tensor_scalar(
                                out=yt[:], in0=lg[:], scalar1=cc[:],
                                scalar2=None, op0=ALU.add)
                            nc.sync.dma_start(
                                out=t_y[b * 128:(b + 1) * 128, :], in_=yt[:])

            # load dstoff per layer once (small)
            dof_sb = sbc.tile([128, nmm_total], f32)
            nc.sync.dma_start(out=dof_sb[:], in_=t_do[:])
            m_base = 0

            if stage >= 3:
                edge_phase(g1_table, g1_bounce, 1)

            if stage >= 4:
                nc.gpsimd.collective_compute(
                    "AllGather", mybir.AluOpType.bypass,
                    replica_groups=[list(range(NCORES))],
                    ins=[g2_bounce[:]], outs=[g2_table[:]])

            if stage >= 5:
                edge_phase(g2_table, g2_bounce, 2)

    nc.compile()
    return nc


def _run(inputs, trace=False):
    import concourse.bass_utils as bass_utils

    x = np.asarray(inputs["x"], np.float32)
    W1 = np.asarray(inputs["W1"], np.float32)
    b1 = np.asarray(inputs["b1"], np.float32)
    W2 = np.asarray(inputs["W2"], np.float32)
    b2 = np.asarray(inputs["b2"], np.float32)

    plan = _plan(x, inputs["edge_index"])
    nc = _build(plan, stage=int(os.environ.get("KSTAGE", "99")))

    b1b = np.tile(b1[None, :], (128, 1)).astype(np.float32)
    b2b = np.tile(b2[None, :], (128, 1)).astype(np.float32)
    W2b = np.tile(W2, (2, 1)).astype(np.float32)
    id2 = np.tile(np.eye(64, dtype=np.float32), (2, 1))

    in_maps = []
    for c in range(NCORES):
        in_maps.append({
            "xT": plan["xT"][c],
            "dis": plan["dis"][c],
            "W1": W1, "W2b": W2b, "b1b": b1b, "b2b": b2b,
            "id2": id2,
            "gidx": plan["gidx"][c],
            "doff": plan["doff"][c],
            "consts": plan["consts"],
        })

    res = bass_utils.run_bass_kernel_spmd(
        nc, in_maps, core_ids=list(range(NCORES)), trace=trace)

    out = np.empty((N, C), np.float32)
    for c in range(NCORES):
        yc = np.asarray(res.results[c]["y"], np.float32)
        nodes_c = plan["nodes_by_core"][c]
        out[nodes_c] = yc[:len(nodes_c)]
    return out, res


def kernel(**inputs):
    out, _ = _run(inputs, trace=False)
    return out


# revision 12
# speedup vs baseline: 1.4259x; 1.2659x over previous
"""GCN 2-layer kernel for Trainium2 (8 NeuronCores, Bass/Tile), v2.

Strategy (vs v1 which used dma_scatter_add into DRAM accumulators):
  - Nodes sharded across 8 cores (degree-balanced snake), S=13312 slots/core
    = 104 blocks of 128 nodes.
  - Per layer: a node table g (g1 = dis*(x@W1), g2 = dis*z) is computed
    per-shard, AllGathered to a full [106496, 64] fp32 DRAM table.
    NOTE: the layer-2 W2 matmul is moved AFTER aggregation (linearity),
    so the layer-2 table is just dis*z.
  - Edge phase: edges (plus self-loops) are grouped by destination block
    (128 dsts) and source range (16K rows, int16 gather index limit), with
    token counts padded to a cross-core-uniform static schedule.
    dma_gather pulls source rows (256B each) into SBUF tiles of
    [128 tokens, 64 feats]; the TensorEngine segment-reduces each tile into
    a feature-major PSUM accumulator [64, 1536] (24-block superblock pairs
    on partition halves) via tile-built 0/1 matrices
    M[token, dst_off] = (dstoff[token] == iota), streamed as matmul rhs.
    PSUM banks are cleared by outer-product zero-matmuls (start=True),
    real matmuls accumulate with start=False.
  - Tails per 128-dst block: transpose to node-major via PE, then
    layer 1: z~ = dis*relu(dis*agg + b1) -> layer-2 table;
    layer 2: h2 = (agg @ W2), y = log_softmax(dis*h2 + b2).
"""
import os
import sys

sys.path.insert(0, "/opt/trn_rl_repo")

import numpy as np

N, FIN, HD, C = 100000, 128, 64, 64
E = 1600000
NCORES = 8
S = 13312                  # slots per core
NT = S // 128              # 104 dst blocks
GROWS = NCORES * S         # 106496 global table rows
RANGE_W = int(os.environ.get("KRW", "16384"))  # gather index range (int16)
NR = (GROWS + RANGE_W - 1) // RANGE_W
PAIRW = 24                 # dst blocks per superblock-pair (2 halves x 12)
NSBP = (NT + PAIRW - 1) // PAIRW          # 5 (24,24,24,24,8)
HALF_BLKS = PAIRW // 2     # 12
HALF_COLS = HALF_BLKS * 128  # 1536 psum cols per half (3 banks)
BATCH = 1024               # max tokens per gather
NQ = 4


def _plan(x, edge_index):
    """Host-side planning. Returns per-core arrays + a uniform schedule."""
    x = np.asarray(x, np.float32)
    ei = np.asarray(edge_index, np.int64)
    src, dst = ei[0], ei[1]

    deg = np.bincount(dst, minlength=N).astype(np.float64) + 1.0  # + self loop
    dis_f = (1.0 / np.sqrt(deg)).astype(np.float32)

    # degree-balanced snake sharding
    order = np.argsort(-deg, kind="stable")
    posn = np.arange(N)
    blk, rem = posn // NCORES, posn % NCORES
    corepat = np.where(blk % 2 == 0, rem, NCORES - 1 - rem).astype(np.int32)
    core_of = np.empty(N, np.int32)
    core_of[order] = corepat

    lrow_of = np.empty(N, np.int64)
    nodes_by_core = []
    for c in range(NCORES):
        nodes_c = order[core_of[order] == c]
        lrow_of[nodes_c] = np.arange(len(nodes_c))
        nodes_by_core.append(nodes_c)

    grow_of = core_of.astype(np.int64) * S + lrow_of

    # per-core token lists: real edges only (self loops get dedicated
    # "self" groups gathering from the local bounce tensor)
    tok_src = []   # global row of source
    tok_dst = []   # local row of dest
    dst_core = core_of[dst]
    for c in range(NCORES):
        m = dst_core == c
        tok_src.append(grow_of[src[m]])
        tok_dst.append(lrow_of[dst[m]])

    # counts per (core, block, range)
    cnt = np.zeros((NCORES, NT, NR), np.int64)
    for c in range(NCORES):
        b = tok_dst[c] >> 7
        r = tok_src[c] // RANGE_W
        np.add.at(cnt[c], (b, r), 1)

    # uniform padded tile counts per (block, range)
    ntok = cnt.max(axis=0)                       # [NT, NR]
    ntiles_br = -(-ntok // 128)                  # ceil to 128-token tiles

    # schedule: for each sbp: a self group (r = -1, one tile per block,
    # gathered from the bounce tensor), then per-range edge groups.
    # Tiles in block-major order; gathers chunk them by 8 (last partial).
    sched = []            # per (sbp, r): dict(tiles=[(b)], ngather)
    total_tiles = 0
    for sbp in range(NSBP):
        b0, b1 = sbp * PAIRW, min((sbp + 1) * PAIRW, NT)
        groups = [(-1, list(range(b0, b1)))]
        for r in range(NR):
            tiles = []
            for b in range(b0, b1):
                tiles += [b] * int(ntiles_br[b, r])
            groups.append((r, tiles))
        for r, tiles in groups:
            ng = -(-len(tiles) // 8) if tiles else 0
            sched.append({
                "sbp": sbp, "r": r, "tiles": tiles, "ngather": ng,
            })
            total_tiles += len(tiles)

    ngather_total = sum(g["ngather"] for g in sched)
    nmm_total = total_tiles

    # build per-core gidx + dstoff arrays following the schedule
    def wrap16(v):
        n = len(v)
        a = np.asarray(v, np.int16).reshape(n // 16, 16).T.copy()
        return np.tile(a, (8, 1))

    gidx_all = np.zeros((NCORES, 128, ngather_total * (BATCH // 16)), np.int16)
    doff_all = np.full((NCORES, 128, max(nmm_total, 1)), -1.0, np.float32)

    for c in range(NCORES):
        b_of = tok_dst[c] >> 7
        r_of = tok_src[c] // RANGE_W
        # bucket tokens by (block, range)
        key = b_of * NR + r_of
        osort = np.argsort(key, kind="stable")
        ts_s, td_s = tok_src[c][osort], tok_dst[c][osort]
        key_s = key[osort]
        starts = np.searchsorted(key_s, np.arange(NT * NR))
        ends = np.searchsorted(key_s, np.arange(NT * NR) + 1)

        g_cursor = 0   # in gathers
        m_cursor = 0   # in mms (= tiles)
        for grp in sched:
            r = grp["r"]
            tiles = grp["tiles"]
            if not tiles:
                continue
            # assemble this group's token stream (tile-major, 128 per tile)
            n_tok = len(tiles) * 128
            idx_loc = np.zeros(n_tok, np.int64)   # gather idx within range
            dof = np.full(n_tok, -1.0, np.float32)
            if r < 0:
                # self group: one tile per block, idx = own local row
                for i, b in enumerate(tiles):
                    idx_loc[i * 128:(i + 1) * 128] = b * 128 + np.arange(128)
                    dof[i * 128:(i + 1) * 128] = np.arange(128, dtype=np.float32)
            else:
                # fill per block
                tpos = 0
                tiles_arr = np.asarray(tiles)
                for b in np.unique(tiles_arr):
                    k = b * NR + r
                    s0, s1 = starts[k], ends[k]
                    cnt_b = s1 - s0
                    nt_b = int(np.count_nonzero(tiles_arr == b))
                    # tokens for this block go into its nt_b tiles
                    off = tpos * 128
                    idx_loc[off:off + cnt_b] = ts_s[s0:s1] - r * RANGE_W
                    dof[off:off + cnt_b] = (
                        td_s[s0:s1] - b * 128).astype(np.float32)
                    tpos += nt_b
            # scatter into gidx (per gather chunk) and dstoff (per tile)
            n_tiles = len(tiles)
            for g in range(grp["ngather"]):
                t0, t1 = g * 8, min((g + 1) * 8, n_tiles)
                nidx = (t1 - t0) * 128
                seg = idx_loc[t0 * 128: t0 * 128 + nidx]
                col0 = (g_cursor + g) * (BATCH // 16)
                w = wrap16(np.pad(seg, (0, BATCH - nidx)))
                gidx_all[c][:, col0:col0 + BATCH // 16] = w
            doff_all[c][:, m_cursor:m_cursor + n_tiles] = (
                dof.reshape(n_tiles, 128).T)
            g_cursor += grp["ngather"]
            m_cursor += n_tiles

    # per-core xT, dis with p = row%128, t = row//128
    xT_all = np.zeros((NCORES, 128, S), np.float32)
    dis_all = np.ones((NCORES, 128, NT), np.float32)
    for c in range(NCORES):
        nodes_c = nodes_by_core[c]
        n_real = len(nodes_c)
        lin = np.arange(n_real)
        xT_all[c][:, lin] = x[nodes_c].T
        dis_all[c][lin % 128, lin // 128] = dis_f[nodes_c]

    # consts tile: [128, 640]: cols 0:128 iota, 128:640 zeros
    consts = np.zeros((128, 640), np.float32)
    consts[:, 0:128] = np.arange(128, dtype=np.float32)[None, :]

    return {
        "sched": sched,
        "ngather_total": ngather_total,
        "nmm_total": nmm_total,
        "gidx": gidx_all,
        "doff": doff_all,
        "xT": xT_all,
        "dis": dis_all,
        "consts": consts,
        "nodes_by_core": nodes_by_core,
    }


def _build(plan, stage=99):
    import concourse.bacc as bacc
    import concourse.bass as bass
    import concourse.tile as tile
    import concourse.mybir as mybir

    f32 = mybir.dt.float32
    i16 = mybir.dt.int16
    AF = mybir.ActivationFunctionType
    ALU = mybir.AluOpType

    sched = plan["sched"]
    ngather_total = plan["ngather_total"]
    nmm_total = plan["nmm_total"]
    gcols = ngather_total * (BATCH // 16)
    max_ng = max(g["ngather"] for g in sched)

    nc = bacc.Bacc("TRN2", target_bir_lowering=False, debug=False,
                   num_devices=NCORES, num_swdge_queues=NQ)

    t_xT = nc.dram_tensor("xT", [128, S], f32, kind="ExternalInput")
    t_dis = nc.dram_tensor("dis", [128, NT], f32, kind="ExternalInput")
    t_W1 = nc.dram_tensor("W1", [FIN, HD], f32, kind="ExternalInput")
    t_W2b = nc.dram_tensor("W2b", [128, C], f32, kind="ExternalInput")
    t_b1 = nc.dram_tensor("b1b", [128, HD], f32, kind="ExternalInput")
    t_b2 = nc.dram_tensor("b2b", [128, C], f32, kind="ExternalInput")
    t_id2 = nc.dram_tensor("id2", [128, 64], f32, kind="ExternalInput")
    t_gi = nc.dram_tensor("gidx", [128, gcols], i16, kind="ExternalInput")
    t_do = nc.dram_tensor("doff", [128, nmm_total], f32, kind="ExternalInput")
    t_cn = nc.dram_tensor("consts", [128, 640], f32, kind="ExternalInput")
    t_y = nc.dram_tensor("y", [S, C], f32, kind="ExternalOutput")

    g1_bounce = nc.dram_tensor("g1_bounce", [S, HD], f32, kind="Internal")
    g2_bounce = nc.dram_tensor("g2_bounce", [S, HD], f32, kind="Internal")
    g1_table = nc.dram_tensor("g1_table", [GROWS, HD], f32, kind="Internal",
                              addr_space="Shared")
    g2_table = nc.dram_tensor("g2_table", [GROWS, HD], f32, kind="Internal",
                              addr_space="Shared")

    with tile.TileContext(nc) as tc:
        with tc.tile_pool(name="sbc", bufs=1) as sbc, \
             tc.tile_pool(name="sbx", bufs=2) as sbx, \
             tc.tile_pool(name="sbg", bufs=6) as sbg, \
             tc.tile_pool(name="sbm", bufs=4) as sbm, \
             tc.tile_pool(name="sbi", bufs=2) as sbi, \
             tc.tile_pool(name="sbt", bufs=3) as sbt, \
             tc.tile_pool(name="psa", bufs=2, space="PSUM") as psa, \
             tc.tile_pool(name="pst", bufs=2, space="PSUM") as pst:

            W1t = sbc.tile([FIN, HD], f32)
            nc.sync.dma_start(out=W1t[:], in_=t_W1[:])
            W2bt = sbc.tile([128, C], f32)
            nc.sync.dma_start(out=W2bt[:], in_=t_W2b[:])
            b1t = sbc.tile([128, HD], f32)
            nc.sync.dma_start(out=b1t[:], in_=t_b1[:])
            b2t = sbc.tile([128, C], f32)
            nc.sync.dma_start(out=b2t[:], in_=t_b2[:])
            id2t = sbc.tile([128, 64], f32)
            nc.sync.dma_start(out=id2t[:], in_=t_id2[:])
            cons = sbc.tile([128, 640], f32)
            nc.sync.dma_start(out=cons[:], in_=t_cn[:])
            dist = sbc.tile([128, NT], f32)
            nc.sync.dma_start(out=dist[:], in_=t_dis[:])

            iota = cons[:, 0:128]
            zrow = cons[0:1, 512:640]      # zeros [1, 128]
            zrhs = cons[0:1, 128:640]      # zeros [1, 512]

            # ---------------- layer-1 prep: g1 = dis * (x @ W1) -------------
            NCHUNK = S // 1024             # 13
            for ch in range(NCHUNK):
                xc = sbx.tile([128, 1024], f32, tag="xc")
                nc.sync.dma_start(out=xc[:],
                                  in_=t_xT[:, ch * 1024:(ch + 1) * 1024])
                for j in range(8):
                    t = 8 * ch + j
                    pp = pst.tile([128, 128], f32, tag="pp")
                    nc.tensor.matmul(out=pp[:, 0:HD],
                                     lhsT=xc[:, j * 128:(j + 1) * 128],
                                     rhs=W1t[:], start=True, stop=True)
                    g1 = sbt.tile([128, HD], f32, tag="g1")
                    nc.vector.tensor_scalar(
                        out=g1[:], in0=pp[:, 0:HD],
                        scalar1=dist[:, t:t + 1], scalar2=None, op0=ALU.mult)
                    nc.sync.dma_start(
                        out=g1_bounce[t * 128:(t + 1) * 128, :], in_=g1[:])

            if stage >= 2:
                nc.gpsimd.collective_compute(
                    "AllGather", mybir.AluOpType.bypass,
                    replica_groups=[list(range(NCORES))],
                    ins=[g1_bounce[:]], outs=[g1_table[:]])

            # ---------------- edge phase ----------------
            def edge_phase(table, bounce, layer):
                g_cursor = 0
                m_cursor = 0
                gq = 0
                for sbp in range(NSBP):
                    b0 = sbp * PAIRW
                    acc = psa.tile([128, HALF_COLS], f32, tag="acc")
                    # clear all 3 banks (both halves at once)
                    for seg in range(3):
                        nc.tensor.matmul(
                            out=acc[:, seg * 512:(seg + 1) * 512],
                            lhsT=zrow[:], rhs=zrhs[:],
                            start=True, stop=False, skip_group_check=True)
                    # find last mm per bank to set stop
                    grp_list = [g for g in sched if g["sbp"] == sbp]
                    last_of_seg = {}
                    mm_idx = 0
                    for grp in grp_list:
                        for b in grp["tiles"]:
                            lb = b - b0
                            seg = ((lb % HALF_BLKS) * 128) // 512
                            last_of_seg[seg] = mm_idx
                            mm_idx += 1
                    mm_idx = 0
                    for grp in grp_list:
                        r = grp["r"]
                        tiles = grp["tiles"]
                        if not tiles:
                            continue
                        if r < 0:
                            src_ap = bounce[:]
                        else:
                            r1 = min((r + 1) * RANGE_W, GROWS)
                            src_ap = table[r * RANGE_W:r1, :]
                        n_tiles = len(tiles)
                        ng = grp["ngather"]
                        gi = sbi.tile([128, max_ng * (BATCH // 16)], i16,
                                      tag="gi")
                        col0 = g_cursor * (BATCH // 16)
                        nc.sync.dma_start(
                            out=gi[:, 0:ng * (BATCH // 16)],
                            in_=t_gi[:, col0:col0 + ng * (BATCH // 16)])
                        for g in range(ng):
                            t0 = g * 8
                            t1 = min(t0 + 8, n_tiles)
                            nt8 = t1 - t0
                            nidx = nt8 * 128
                            gcol = g * (BATCH // 16)
                            buf = sbg.tile([128, 8, HD], f32, tag="buf")
                            nc.gpsimd.dma_gather(
                                out_ap=buf[:, 0:nt8, :],
                                in_ap=src_ap,
                                idxs_ap=gi[:, gcol:gcol + nidx // 16],
                                num_idxs=nidx,
                                num_idxs_reg=nidx,
                                elem_size=HD,
                                queue_num=gq % NQ,
                                single_packet=os.environ.get("KSP", "1") == "1",
                            )
                            gq += 1
                            # batched M build for this chunk's tiles
                            mm0 = m_cursor + t0
                            M = sbm.tile([128, 8, 128], f32, tag="M")
                            do_sl = dof_sb[:, mm0 - m_base:mm0 - m_base + nt8]
                            nc.vector.tensor_tensor(
                                out=M[:, 0:nt8, :],
                                in0=iota.unsqueeze(1).broadcast_to(
                                    [128, nt8, 128]),
                                in1=do_sl.unsqueeze(2).broadcast_to(
                                    [128, nt8, 128]),
                                op=ALU.is_equal)
                            for j in range(nt8):
                                b = tiles[t0 + j]
                                lb = b - b0
                                half = lb // HALF_BLKS
                                col = (lb % HALF_BLKS) * 128
                                nc.tensor.matmul(
                                    out=acc[64 * half:64 * half + 64,
                                            col:col + 128],
                                    lhsT=buf[:, j, :],
                                    rhs=M[:, j, :],
                                    start=False,
                                    stop=(mm_idx == last_of_seg.get(
                                        (col // 512), -2)),
                                    skip_group_check=True,
                                    tile_position=(0, 64 * half))
                                mm_idx += 1
                        g_cursor += grp["ngather"]
                        m_cursor += n_tiles
                    # ---------------- tails for this sbp ----------------
                    nblk = min(PAIRW, NT - b0)
                    for lb in range(nblk):
                        b = b0 + lb
                        half = lb // HALF_BLKS
                        col = (lb % HALF_BLKS) * 128
                        hs, he = 64 * half, 64 * half + 64
                        pt = sbt.tile([128, 128], f32, tag="pt")
                        nc.scalar.copy(pt[hs:he, :], acc[hs:he, col:col + 128])
                        if layer == 1:
                            tr = pst.tile([128, 128], f32, tag="pp")
                            nc.tensor.transpose(out=tr[:, 0:64],
                                                in_=pt[hs:he, :],
                                                identity=id2t[hs:he, :])
                            t1v = sbt.tile([128, HD], f32, tag="t1v")
                            nc.vector.scalar_tensor_tensor(
                                out=t1v[:], in0=tr[:, 0:64],
                                scalar=dist[:, b:b + 1],
                                in1=b1t[:], op0=ALU.mult, op1=ALU.add)
                            zt = sbt.tile([128, HD], f32, tag="zt")
                            nc.scalar.activation(zt[:], t1v[:], AF.Relu)
                            zs = sbt.tile([128, HD], f32, tag="zs")
                            nc.vector.tensor_scalar(
                                out=zs[:], in0=zt[:],
                                scalar1=dist[:, b:b + 1], scalar2=None,
                                op0=ALU.mult)
                            nc.sync.dma_start(
                                out=g2_bounce[b * 128:(b + 1) * 128, :],
                                in_=zs[:])
                        else:
                            h2T = pst.tile([128, 128], f32, tag="pp")
                            nc.tensor.matmul(out=h2T[0:64, :],
                                             lhsT=W2bt[hs:he, :],
                                             rhs=pt[hs:he, :],
                                             start=True, stop=True,
                                             tile_position=(64 * half, 0))
                            h2Ts = sbt.tile([128, 128], f32, tag="h2Ts")
                            nc.scalar.copy(h2Ts[0:64, :], h2T[0:64, :])
                            h2 = pst.tile([128, 128], f32, tag="pp")
                            nc.tensor.transpose(out=h2[:, 0:64],
                                                in_=h2Ts[0:64, :],
                                                identity=id2t[0:64, :])
                            lg = sbt.tile([128, C], f32, tag="lg")
                            nc.vector.scalar_tensor_tensor(
                                out=lg[:], in0=h2[:, 0:64],
                                scalar=dist[:, b:b + 1],
                                in1=b2t[:], op0=ALU.mult, op1=ALU.add)
                            nmax = sbt.tile([128, 1], f32, tag="nmax")
                            nc.vector.tensor_reduce(
                                out=nmax[:], in_=lg[:],
                                axis=mybir.AxisListType.X,
                                op=ALU.max, negate=True)
                            ex = sbt.tile([128, C], f32, tag="ex")
                            sume = sbt.tile([128, 1], f32, tag="sume")
                            nc.scalar.activation(ex[:], lg[:], AF.Exp,
                                                 bias=nmax[:], scale=1.0,
                                                 accum_out=sume[:])
                            lse = sbt.tile([128, 1], f32, tag="lse")
                            nc.scalar.activation(lse[:], sume[:], AF.Ln)
                            cc = sbt.tile([128, 1], f32, tag="cc")
                            nc.vector.tensor_tensor(out=cc[:], in0=nmax[:],
                                                    in1=lse[:],
                                                    op=ALU.subtract)
                            yt = sbt.tile([128, C], f32, tag="yt")
                            nc.vector.tensor_scalar(
                                out=yt[:], in0=lg[:], scalar1=cc[:],
                                scalar2=None, op0=ALU.add)
                            nc.sync.dma_start(
                                out=t_y[b * 128:(b + 1) * 128, :], in_=yt[:])

            # load dstoff per layer once (small)
            dof_sb = sbc.tile([128, nmm_total], f32)
            nc.sync.dma_start(out=dof_sb[:], in_=t_do[:])
            m_base = 0

            if stage >= 3:
                edge_phase(g1_table, g1_bounce, 1)

            if stage >= 4:
                nc.gpsimd.collective_compute(
                    "AllGather", mybir.AluOpType.bypass,
                    replica_groups=[list(range(NCORES))],
                    ins=[g2_bounce[:]], outs=[g2_table[:]])

            if stage >= 5:
                edge_phase(g2_table, g2_bounce, 2)

    nc.compile()
    return nc


def _run(inputs, trace=False):
    import concourse.bass_utils as bass_utils

    x = np.asarray(inputs["x"], np.float32)
    W1 = np.asarray(inputs["W1"], np.float32)
    b1 = np.asarray(inputs["b1"], np.float32)
    W2 = np.asarray(inputs["W2"], np.float32)
    b2 = np.asarray(inputs["b2"], np.float32)

    plan = _plan(x, inputs["edge_index"])
    nc = _build(plan, stage=int(os.environ.get("KSTAGE", "99")))

    b1b = np.tile(b1[None, :], (128, 1)).astype(np.float32)
    b2b = np.tile(b2[None, :], (128, 1)).astype(np.float32)
    W2b = np.tile(W2, (2, 1)).astype(np.float32)
    id2 = np.tile(np.eye(64, dtype=np.float32), (2, 1))

    in_maps = []
    for c in range(NCORES):
        in_maps.append({
            "xT": plan["xT"][c],
            "dis": plan["dis"][c],
            "W1": W1, "W2b": W2b, "b1b": b1b, "b2b": b2b,
            "id2": id2,
            "gidx": plan["gidx"][c],
            "doff": plan["doff"][c],
            "consts": plan["consts"],
        })

    res = bass_utils.run_bass_kernel_spmd(
        nc, in_maps, core_ids=list(range(NCORES)), trace=trace)

    out = np.empty((N, C), np.float32)
    for c in range(NCORES):
        yc = np.asarray(res.results[c]["y"], np.float32)
        nodes_c = plan["nodes_by_core"][c]
        out[nodes_c] = yc[:len(nodes_c)]
    return out, res


def kernel(**inputs):
    out, _ = _run(inputs, trace=False)
    return out
